# revision 11
# baseline (speedup 1.0000x reference)
"""CapsuleLayer (B=32, J=32, I=2048, T=16, D=16, 3 routing iters) on 8 TRN2 cores.

Strategy: shard input-capsule axis I across the 8 cores (I_loc = 256).
W reads stay at the 8.4 MB/core roofline; all routing state except the
tiny s[b,j,t] (64 KB, AllReduce x3) is core-local.

v3 (from trace analysis of the 369 us baseline and the v2 attempt):
  - warmup dummy AllReduce at t=0 absorbs the ~35 us first-collective
    setup/skew that serialized AR0.
  - phase A: W tiles streamed once; s0 folded out of the same W tiles
    via a 3rd accumulating matmul per tile (lhsT = x/J, K=128), killing
    the separate 64-matmul restream pass.  W columns host-laid as (t,j)
    so PSUM->SBUF copies are straight, alternating ACT/DVE.
  - routing: explicit 3-chunk-deep software pipeline.  The tile
    scheduler keeps per-engine program order, so cross-engine offload
    (l1 tree level + c-normalize on GpSimd, per-group exp with
    accum_out z on ACT) only overlaps when the emission order is
    interleaved by hand.
"""

import functools
import os
import sys

import numpy as np

sys.path.insert(0, "/opt/trn_rl_repo")

import ml_dtypes  # noqa: E402

import concourse.bass as bass  # noqa: E402
import concourse.bacc as bacc  # noqa: E402
import concourse.mybir as mybir  # noqa: E402
import concourse.tile as tile  # noqa: E402

F32 = mybir.dt.float32
F32R = mybir.dt.float32r
BF16 = mybir.dt.bfloat16

NCORES = 8
B, J, I, T, D = 32, 32, 2048, 16, 16
ILOC = I // NCORES          # 256
G = ILOC // 4               # 64 i-groups of 4
NWT = G // 2                # 32 W dram tiles [128, 512], 2 groups each
EPS = 1e-9
SKEW = 3                    # software-pipeline depth (chunks in flight)


def _build_program(single=False):
    nc = bacc.Bacc(
        "TRN2",
        target_bir_lowering=False,
        debug=False,
        enable_asserts=False,
        num_devices=1 if single else NCORES,
    )

    wt_d = nc.dram_tensor("wt", [NWT, 128, 512], F32R, kind="ExternalInput")
    xd_d = nc.dram_tensor("xd", [NWT, 128, 128], F32R, kind="ExternalInput")
    xs_d = nc.dram_tensor("xs0", [NWT, 128, 32], F32R, kind="ExternalInput")
    ones_d = nc.dram_tensor("onesdb", [128, 32], BF16, kind="ExternalInput")
    out_d = nc.dram_tensor("outv", [32, 512], F32, kind="ExternalOutput")

    with tile.TileContext(nc) as tc:
        _capsule(
            tc, wt_d.ap(), xd_d.ap(), xs_d.ap(), ones_d.ap(), out_d.ap(),
            single=single,
        )
    nc.compile()
    return nc


def _capsule(tc, wt, xd, xs0d, ones_dram, outv, single=False):
    nc = tc.nc
    from contextlib import ExitStack

    ctx = ExitStack()
    with ctx:
        up = ctx.enter_context(tc.tile_pool(name="u", bufs=1))
        wp = ctx.enter_context(tc.tile_pool(name="w", bufs=4))
        xp = ctx.enter_context(tc.tile_pool(name="x", bufs=1))
        cp = ctx.enter_context(tc.tile_pool(name="consts", bufs=1))
        qp = ctx.enter_context(tc.tile_pool(name="q", bufs=4))
        l1p = ctx.enter_context(tc.tile_pool(name="l1", bufs=2))
        tp = ctx.enter_context(tc.tile_pool(name="tree", bufs=3))
        bp = ctx.enter_context(tc.tile_pool(name="bij", bufs=1))
        sp = ctx.enter_context(tc.tile_pool(name="small", bufs=2))
        vp = ctx.enter_context(tc.tile_pool(name="vexp", bufs=2))
        pup = ctx.enter_context(tc.tile_pool(name="upsum", bufs=4, space="PSUM"))
        s0p = ctx.enter_context(tc.tile_pool(name="s0psum", bufs=1, space="PSUM"))
        psp = ctx.enter_context(tc.tile_pool(name="spsum", bufs=2, space="PSUM"))
        dp = ctx.enter_context(tc.tile_pool(name="dram", bufs=8, space="DRAM"))

        # ---- persistent tiles
        u = up.tile([128, G * 512], BF16)          # [(i_sub,b), (g,t,j)]
        xall = xp.tile([128, NWT * 128], F32R)     # block-diag x weights
        xsum = xp.tile([128, NWT * 32], F32R)      # x/J columns for s0
        ones = cp.tile([128, 32], BF16)            # delta_b
        bijs = [bp.tile([128, 256], F32, tag=f"bij{c}", name=f"bij{c}") for c in range(8)]

        # DMA order tuned so the first matmul's deps (xall chunk 0,
        # xsum, w0) land first.
        nc.sync.dma_start(
            xall[:, 0:1024].rearrange("k (p m) -> k p m", p=8),
            xd[0:8].transpose([1, 0, 2]),
        )
        nc.sync.dma_start(
            xsum[:, :].rearrange("k (p m) -> k p m", p=NWT),
            xs0d.transpose([1, 0, 2]),
        )

        # ---- phase A: u_hat + s0 in one W pass
        s0ps = s0p.tile([32, 512], F32)
        copy_engs = [nc.scalar.copy, nc.vector.tensor_copy]
        for p in range(NWT):
            w = wp.tile([128, 512], F32R, tag="w")
            nc.sync.dma_start(w[:, :], wt[p])
            if p == 0:
                # remaining low-urgency input DMAs, queued behind w0
                nc.sync.dma_start(ones[:, :], ones_dram)
                for c in range(1, 4):
                    nc.sync.dma_start(
                        xall[:, c * 1024 : (c + 1) * 1024].rearrange(
                            "k (p m) -> k p m", p=8
                        ),
                        xd[c * 8 : (c + 1) * 8].transpose([1, 0, 2]),
                    )
            for gl in range(2):
                g = 2 * p + gl
                ups = pup.tile([128, 512], F32)
                nc.tensor.matmul(
                    ups[:, :],
                    lhsT=xall[gl * 64 : (gl + 1) * 64, p * 128 : (p + 1) * 128],
                    rhs=w[gl * 64 : (gl + 1) * 64, :],
                    start=True,
                    stop=True,
                )
                # psum free = (t,j) ; straight copy, rotating engines
                copy_engs[g % 2](u[:, g * 512 : (g + 1) * 512], ups[:, :])
            # s0 partial: full-K contraction of x/J against the same tile
            nc.tensor.matmul(
                s0ps[:, :],
                lhsT=xsum[:, p * 32 : (p + 1) * 32],
                rhs=w[:, :],
                start=(p == 0),
                stop=(p == NWT - 1),
                skip_group_check=True,
            )

        v_r = _allreduce_squash(tc, dp, sp, s0ps, r=0, single=single)

        # ---- routing iterations: software-pipelined chunk loop
        for r in (1, 2):
            vexp = vp.tile([128, 512], BF16, tag="vexp")
            for k in range(4):
                nc.sync.dma_start(vexp[k * 32 : (k + 1) * 32, :], v_r[:, :])

            sps = psp.tile([32, 512], F32, tag="s")
            ctes = [None] * 8

            def front(ch):
                """q = u*v, full logits tree, bij update, exp.  All DVE
                except the exp (ACT); back() of the previous chunk slots
                into DVE's stream while ACT runs."""
                bij = bijs[ch]
                usl = u[:, ch * 4096 : (ch + 1) * 4096].rearrange(
                    "p (g t j) -> p g t j", g=8, t=16
                )
                q = qp.tile([128, 4096], BF16, tag="q")
                vb = (
                    vexp[:, :]
                    .rearrange("p (t j) -> p t j", t=16)
                    .unsqueeze(1)
                    .to_broadcast([128, 8, 16, 32])
                )
                nc.vector.tensor_mul(
                    q[:, :].rearrange("p (g t j) -> p g t j", g=8, t=16), usl, vb
                )
                l1 = l1p.tile([128, 2048], BF16, tag="l1")
                q4 = q[:, :].rearrange("p (g t j) -> p g t j", g=8, t=16)
                nc.vector.tensor_add(
                    l1[:, :].rearrange("p (g t j) -> p g t j", g=8, t=8),
                    q4[:, :, 0:8, :],
                    q4[:, :, 8:16, :],
                )
                l2 = tp.tile([128, 1024], BF16, tag="l2")
                l14 = l1[:, :].rearrange("p (g t j) -> p g t j", g=8, t=8)
                nc.vector.tensor_add(
                    l2[:, :].rearrange("p (g t j) -> p g t j", g=8, t=4),
                    l14[:, :, 0:4, :],
                    l14[:, :, 4:8, :],
                )
                l3 = tp.tile([128, 512], BF16, tag="l3")
                l24 = l2[:, :].rearrange("p (g t j) -> p g t j", g=8, t=4)
                nc.vector.tensor_add(
                    l3[:, :].rearrange("p (g t j) -> p g t j", g=8, t=2),
                    l24[:, :, 0:2, :],
                    l24[:, :, 2:4, :],
                )
                bsl = bij[:, :].rearrange("p (g j) -> p g j", g=8)
                l3a = l3[:, :].rearrange("p (g t j) -> p g t j", g=8, t=2)
                if r == 1:
                    nc.vector.tensor_add(bsl, l3a[:, :, 0, :], l3a[:, :, 1, :])
                else:
                    dd = tp.tile([128, 256], F32, tag="dd")
                    nc.vector.tensor_add(
                        dd[:, :].rearrange("p (g j) -> p g j", g=8),
                        l3a[:, :, 0, :],
                        l3a[:, :, 1, :],
                    )
                    nc.vector.tensor_add(
                        bsl, bsl, dd[:, :].rearrange("p (g j) -> p g j", g=8)
                    )
                cte = tp.tile([128, 256], BF16, tag="cte")
                nc.scalar.activation(
                    cte[:, :], bij[:, :], mybir.ActivationFunctionType.Exp
                )
                ctes[ch] = cte

            def back(ch):
                """Normalize, p2 = c*u, PE restream.  All DVE."""
                usl = u[:, ch * 4096 : (ch + 1) * 4096].rearrange(
                    "p (g t j) -> p g t j", g=8, t=16
                )
                cte = ctes[ch]
                z = tp.tile([128, 8], F32, tag="z")
                nc.vector.tensor_reduce(
                    z[:, :],
                    cte[:, :].rearrange("p (g j) -> p g j", g=8),
                    mybir.AxisListType.X,
                    mybir.AluOpType.add,
                )
                invz = tp.tile([128, 8], F32, tag="invz")
                nc.vector.reciprocal(invz[:, :], z[:, :])
                cc = tp.tile([128, 256], BF16, tag="cc")
                nc.vector.tensor_mul(
                    cc[:, :].rearrange("p (g j) -> p g j", g=8),
                    cte[:, :].rearrange("p (g j) -> p g j", g=8),
                    invz[:, :].unsqueeze(2).to_broadcast([128, 8, 32]),
                )

                # op2: p2 = c * u ; PE restream accumulates s
                p2 = qp.tile([128, 4096], BF16, tag="q")
                ccb = (
                    cc[:, :]
                    .rearrange("p (g j) -> p g j", g=8)
                    .unsqueeze(2)
                    .to_broadcast([128, 8, 16, 32])
                )
                nc.vector.tensor_mul(
                    p2[:, :].rearrange("p (g t j) -> p g t j", g=8, t=16), usl, ccb
                )
                for gl in range(8):
                    g = ch * 8 + gl
                    nc.tensor.matmul(
                        sps[:, :],
                        lhsT=ones[:, :],
                        rhs=p2[:, gl * 512 : (gl + 1) * 512],
                        start=(g == 0),
                        stop=(g == G - 1),
                    )

            for ch in range(9):
                if ch < 8:
                    front(ch)
                if ch >= 1:
                    back(ch - 1)

            v_r = _allreduce_squash(tc, dp, sp, sps, r=r, single=single)

        nc.sync.dma_start(outv, v_r[:, :])


def _allreduce_squash(tc, dp, sp, s_psum, r, single=False):
    """AllReduce s [32,512] across cores, then v = squash(s).

    s_psum is the PSUM accumulator.  Returns the v tile (bf16 for r<2,
    f32 for the final r=2).
    """
    nc = tc.nc
    spart = sp.tile([32, 512], F32, tag="s_part")
    nc.scalar.copy(spart[:, :], s_psum[:, :])
    ccin = dp.tile([32, 512], F32, tag=f"ccin{r}")
    ccout = dp.tile([32, 512], F32, tag=f"ccout{r}")
    nc.sync.dma_start(ccin[:, :], spart[:, :])
    if single:
        nc.sync.dma_start(ccout[:, :], ccin[:, :])
    else:
        nc.gpsimd.collective_compute(
            "AllReduce",
            mybir.AluOpType.add,
            replica_groups=[list(range(NCORES))],
            ins=[ccin[:, :].opt()],
            outs=[ccout[:, :].opt()],
        )
    s = sp.tile([32, 512], F32, tag="s_full")
    nc.sync.dma_start(s[:, :], ccout[:, :])

    # squash: v = s * (|s|^2/(1+|s|^2)/sqrt(|s|^2+eps)) per (b,j), |.| over t
    sq = sp.tile([32, 512], F32, tag="sq")
    nc.vector.tensor_mul(sq[:, :], s[:, :], s[:, :])
    ssq = sp.tile([32, 32], F32, tag="ssq")
    nc.vector.tensor_reduce(
        ssq[:, :],
        sq[:, :].rearrange("p (t j) -> p j t", t=16),
        mybir.AxisListType.X,
        mybir.AluOpType.add,
    )
    t1 = sp.tile([32, 32], F32, tag="t1")
    nc.vector.tensor_scalar_add(t1[:, :], ssq[:, :], 1.0)
    r1 = sp.tile([32, 32], F32, tag="r1")
    nc.vector.reciprocal(r1[:, :], t1[:, :])
    ssqe = sp.tile([32, 32], F32, tag="ssqe")
    nc.vector.tensor_scalar_add(ssqe[:, :], ssq[:, :], EPS)
    t2 = sp.tile([32, 32], F32, tag="t2")
    nc.scalar.activation(
        t2[:, :], ssqe[:, :], mybir.ActivationFunctionType.Sqrt, bias=0.0
    )
    r2 = sp.tile([32, 32], F32, tag="r2")
    nc.vector.reciprocal(r2[:, :], t2[:, :])
    sc = sp.tile([32, 32], F32, tag="sc")
    nc.vector.tensor_mul(sc[:, :], ssq[:, :], r1[:, :])
    nc.vector.tensor_mul(sc[:, :], sc[:, :], r2[:, :])
    v = sp.tile([32, 512], F32 if r == 2 else BF16, tag=f"v{r}")
    nc.vector.tensor_mul(
        v[:, :].rearrange("p (t j) -> p t j", t=16),
        s[:, :].rearrange("p (t j) -> p t j", t=16),
        sc[:, :].unsqueeze(1).to_broadcast([32, 16, 32]),
    )
    return v


@functools.lru_cache(maxsize=2)
def _get_nc(single=False):
    return _build_program(single=single)


def _prep_inputs(inputs, W):
    """Build per-core input maps (host-side layout only)."""
    inputs = np.asarray(inputs, dtype=np.float32)
    W = np.asarray(W, dtype=np.float32)
    W0 = W[0]  # [J, I, T, D]

    # delta_b ones [K=(i_sub 4, b 32), M=(b' 32)]
    ones = np.zeros((4, 32, 32), dtype=np.float32)
    for b in range(32):
        ones[:, b, b] = 1.0
    ones = ones.reshape(128, 32).astype(ml_dtypes.bfloat16)

    in_maps = []
    for c in range(NCORES):
        isl = slice(c * ILOC, (c + 1) * ILOC)
        ws = W0[:, isl]  # [J, 256, T, D]
        # wt[p, (gl, i_sub, d), (t, j)] ; i = 8p + 4gl + i_sub
        A = ws.transpose(1, 3, 2, 0)  # [i, d, t, j]
        A = A.reshape(NWT, 2, 4, D, T, J)  # p, gl, i_sub, d, t, j
        wtc = np.ascontiguousarray(A.reshape(NWT, 128, T * J))

        xs = inputs[:, isl]  # [b, 256, d]
        xt = xs.transpose(1, 2, 0)  # [i, d, b]
        xt5 = xt.reshape(NWT, 2, 4, D, B)  # p, gl, i_sub, d, b
        xdc = np.zeros((NWT, 2, 4, D, 4, B), dtype=np.float32)
        ar = np.arange(4)
        # advanced indexing: result axes [i_sub, p, gl, d, b]
        xdc[:, :, ar, :, ar, :] = xt5.transpose(2, 0, 1, 3, 4)
        xdc = np.ascontiguousarray(xdc.reshape(NWT, 128, 128))

        # s0 weights: x/J stacked over the full (gl, i_sub, d) partition
        xsc = np.ascontiguousarray((xt5 / float(J)).reshape(NWT, 128, B))

        in_maps.append({"wt": wtc, "xd": xdc, "xs0": xsc, "onesdb": ones})
    return in_maps


def kernel(inputs, W):
    import concourse.bass_utils as bass_utils

    nc = _get_nc()
    in_maps = _prep_inputs(inputs, W)
    res = bass_utils.run_bass_kernel_spmd(nc, in_maps, list(range(NCORES)))
    v = np.asarray(res.results[0]["outv"])  # [32, 512] = [b, (t, j)]
    return np.ascontiguousarray(
        v.reshape(B, T, J).transpose(0, 2, 1)
    ).astype(np.float32)


# revision 16
# speedup vs baseline: 1.2256x; 1.2256x over previous
"""CapsuleLayer (B=32, J=32, I=2048, T=16, D=16, 3 routing iters) on 8 TRN2 cores.

Strategy: shard input-capsule axis I across the 8 cores (I_loc = 256).
W reads stay at the 8.4 MB/core roofline; all routing state except the
tiny s[b,j,t] (64 KB, AllReduce x3) is core-local.

v3 (from trace analysis of the 369 us baseline and the v2 attempt):
  - warmup dummy AllReduce at t=0 absorbs the ~35 us first-collective
    setup/skew that serialized AR0.
  - phase A: W tiles streamed once; s0 folded out of the same W tiles
    via a 3rd accumulating matmul per tile (lhsT = x/J, K=128), killing
    the separate 64-matmul restream pass.  W columns host-laid as (t,j)
    so PSUM->SBUF copies are straight, alternating ACT/DVE.
  - routing: explicit 3-chunk-deep software pipeline.  The tile
    scheduler keeps per-engine program order, so cross-engine offload
    (l1 tree level + c-normalize on GpSimd, per-group exp with
    accum_out z on ACT) only overlaps when the emission order is
    interleaved by hand.
"""

import functools
import os
import sys

import numpy as np

sys.path.insert(0, "/opt/trn_rl_repo")

import ml_dtypes  # noqa: E402

import concourse.bass as bass  # noqa: E402
import concourse.bacc as bacc  # noqa: E402
import concourse.mybir as mybir  # noqa: E402
import concourse.tile as tile  # noqa: E402

F32 = mybir.dt.float32
F32R = mybir.dt.float32r
BF16 = mybir.dt.bfloat16

NCORES = 8
B, J, I, T, D = 32, 32, 2048, 16, 16
ILOC = I // NCORES          # 256
G = ILOC // 4               # 64 i-groups of 4
NWT = G // 2                # 32 W dram tiles [128, 512], 2 groups each
EPS = 1e-9
SKEW = 3                    # software-pipeline depth (chunks in flight)


def _build_program(single=False):
    nc = bacc.Bacc(
        "TRN2",
        target_bir_lowering=False,
        debug=False,
        enable_asserts=False,
        num_devices=1 if single else NCORES,
    )

    wt_d = nc.dram_tensor("wt", [NWT, 128, 512], BF16, kind="ExternalInput")
    xd_d = nc.dram_tensor("xd", [NWT, 128, 128], BF16, kind="ExternalInput")
    xs_d = nc.dram_tensor("xs0", [NWT, 128, 32], BF16, kind="ExternalInput")
    ones_d = nc.dram_tensor("onesdb", [128, 32], BF16, kind="ExternalInput")
    out_d = nc.dram_tensor("outv", [32, 512], F32, kind="ExternalOutput")

    with tile.TileContext(nc) as tc:
        _capsule(
            tc, wt_d.ap(), xd_d.ap(), xs_d.ap(), ones_d.ap(), out_d.ap(),
            single=single,
        )
    nc.compile()
    return nc


def _capsule(tc, wt, xd, xs0d, ones_dram, outv, single=False):
    nc = tc.nc
    from contextlib import ExitStack

    ctx = ExitStack()
    with ctx:
        up = ctx.enter_context(tc.tile_pool(name="u", bufs=1))
        wp = ctx.enter_context(tc.tile_pool(name="w", bufs=1))
        xp = ctx.enter_context(tc.tile_pool(name="x", bufs=1))
        cp = ctx.enter_context(tc.tile_pool(name="consts", bufs=1))
        qp = ctx.enter_context(tc.tile_pool(name="q", bufs=4))
        l1p = ctx.enter_context(tc.tile_pool(name="l1", bufs=2))
        tp = ctx.enter_context(tc.tile_pool(name="tree", bufs=3))
        bp = ctx.enter_context(tc.tile_pool(name="bij", bufs=1))
        sp = ctx.enter_context(tc.tile_pool(name="small", bufs=2))
        vp = ctx.enter_context(tc.tile_pool(name="vexp", bufs=2))
        pup = ctx.enter_context(tc.tile_pool(name="upsum", bufs=4, space="PSUM"))
        s0p = ctx.enter_context(tc.tile_pool(name="s0psum", bufs=1, space="PSUM"))
        psp = ctx.enter_context(tc.tile_pool(name="spsum", bufs=2, space="PSUM"))
        dp = ctx.enter_context(tc.tile_pool(name="dram", bufs=8, space="DRAM"))

        # ---- persistent tiles
        u = up.tile([128, G * 512], BF16)          # [(i_sub,b), (g,t,j)]
        wall = wp.tile([128, NWT * 512], BF16)     # all W tiles, resident
        xall = xp.tile([128, NWT * 128], BF16)     # block-diag x weights
        xsum = xp.tile([128, NWT * 32], BF16)      # x/J columns for s0
        ones = cp.tile([128, 32], BF16)            # delta_b
        bijs = [bp.tile([128, 256], F32, tag=f"bij{c}", name=f"bij{c}") for c in range(8)]

        # xsum first: the s0 matmuls are the critical path to AR0.
        nc.sync.dma_start(
            xsum[:, :].rearrange("k (p m) -> k p m", p=NWT),
            xs0d.transpose([1, 0, 2]),
        )

        # ---- phase A1: s0 only, one accumulating matmul per W tile.
        # This is all that gates AR0, so the collective fires ~30 us
        # into each core's timeline; the launch skew and the AllReduce
        # then overlap with the u_hat matmuls of phase A2.
        s0ps = s0p.tile([32, 512], F32)
        for p in range(NWT):
            wsl = wall[:, p * 512 : (p + 1) * 512]
            nc.sync.dma_start(wsl, wt[p])
            if p == 0:
                nc.sync.dma_start(
                    xall[:, :].rearrange("k (p m) -> k p m", p=NWT),
                    xd.transpose([1, 0, 2]),
                )
                nc.sync.dma_start(ones[:, :], ones_dram)
            nc.tensor.matmul(
                s0ps[:, :],
                lhsT=xsum[:, p * 32 : (p + 1) * 32],
                rhs=wsl,
                start=(p == 0),
                stop=(p == NWT - 1),
                skip_group_check=True,
            )

        v_r = _allreduce_squash(tc, dp, sp, s0ps, r=0, single=single)

        # ---- phase A2: u_hat from the resident W tiles (overlaps the
        # launch-skew-dominated AR0).
        copy_engs = [nc.scalar.copy, nc.vector.tensor_copy]
        for p in range(NWT):
            for gl in range(2):
                g = 2 * p + gl
                ups = pup.tile([128, 512], F32)
                nc.tensor.matmul(
                    ups[:, :],
                    lhsT=xall[gl * 64 : (gl + 1) * 64, p * 128 : (p + 1) * 128],
                    rhs=wall[gl * 64 : (gl + 1) * 64, p * 512 : (p + 1) * 512],
                    start=True,
                    stop=True,
                )
                # psum free = (t,j) ; straight copy, rotating engines
                copy_engs[g % 2](u[:, g * 512 : (g + 1) * 512], ups[:, :])

        # ---- routing iterations: software-pipelined chunk loop
        for r in (1, 2):
            vexp = vp.tile([128, 512], BF16, tag="vexp")
            for k in range(4):
                nc.sync.dma_start(vexp[k * 32 : (k + 1) * 32, :], v_r[:, :])

            sps = psp.tile([32, 512], F32, tag="s")
            ctes = [None] * 8

            def front(ch):
                """q = u*v, full logits tree, bij update, exp.  All DVE
                except the exp (ACT); back() of the previous chunk slots
                into DVE's stream while ACT runs."""
                bij = bijs[ch]
                usl = u[:, ch * 4096 : (ch + 1) * 4096].rearrange(
                    "p (g t j) -> p g t j", g=8, t=16
                )
                q = qp.tile([128, 4096], BF16, tag="q")
                vb = (
                    vexp[:, :]
                    .rearrange("p (t j) -> p t j", t=16)
                    .unsqueeze(1)
                    .to_broadcast([128, 8, 16, 32])
                )
                nc.vector.tensor_mul(
                    q[:, :].rearrange("p (g t j) -> p g t j", g=8, t=16), usl, vb
                )
                l1 = l1p.tile([128, 2048], BF16, tag="l1")
                q4 = q[:, :].rearrange("p (g t j) -> p g t j", g=8, t=16)
                nc.vector.tensor_add(
                    l1[:, :].rearrange("p (g t j) -> p g t j", g=8, t=8),
                    q4[:, :, 0:8, :],
                    q4[:, :, 8:16, :],
                )
                l2 = tp.tile([128, 1024], BF16, tag="l2")
                l14 = l1[:, :].rearrange("p (g t j) -> p g t j", g=8, t=8)
                nc.vector.tensor_add(
                    l2[:, :].rearrange("p (g t j) -> p g t j", g=8, t=4),
                    l14[:, :, 0:4, :],
                    l14[:, :, 4:8, :],
                )
                l3 = tp.tile([128, 512], BF16, tag="l3")
                l24 = l2[:, :].rearrange("p (g t j) -> p g t j", g=8, t=4)
                nc.vector.tensor_add(
                    l3[:, :].rearrange("p (g t j) -> p g t j", g=8, t=2),
                    l24[:, :, 0:2, :],
                    l24[:, :, 2:4, :],
                )
                bsl = bij[:, :].rearrange("p (g j) -> p g j", g=8)
                l3a = l3[:, :].rearrange("p (g t j) -> p g t j", g=8, t=2)
                if r == 1:
                    nc.vector.tensor_add(bsl, l3a[:, :, 0, :], l3a[:, :, 1, :])
                else:
                    dd = tp.tile([128, 256], F32, tag="dd")
                    nc.vector.tensor_add(
                        dd[:, :].rearrange("p (g j) -> p g j", g=8),
                        l3a[:, :, 0, :],
                        l3a[:, :, 1, :],
                    )
                    nc.vector.tensor_add(
                        bsl, bsl, dd[:, :].rearrange("p (g j) -> p g j", g=8)
                    )
                cte = tp.tile([128, 256], BF16, tag="cte")
                nc.scalar.activation(
                    cte[:, :], bij[:, :], mybir.ActivationFunctionType.Exp
                )
                ctes[ch] = cte

            def back(ch):
                """Normalize, p2 = c*u, PE restream.  All DVE."""
                usl = u[:, ch * 4096 : (ch + 1) * 4096].rearrange(
                    "p (g t j) -> p g t j", g=8, t=16
                )
                cte = ctes[ch]
                z = tp.tile([128, 8], F32, tag="z")
                nc.vector.tensor_reduce(
                    z[:, :],
                    cte[:, :].rearrange("p (g j) -> p g j", g=8),
                    mybir.AxisListType.X,
                    mybir.AluOpType.add,
                )
                invz = tp.tile([128, 8], F32, tag="invz")
                nc.vector.reciprocal(invz[:, :], z[:, :])
                cc = tp.tile([128, 256], BF16, tag="cc")
                nc.vector.tensor_mul(
                    cc[:, :].rearrange("p (g j) -> p g j", g=8),
                    cte[:, :].rearrange("p (g j) -> p g j", g=8),
                    invz[:, :].unsqueeze(2).to_broadcast([128, 8, 32]),
                )

                # op2: p2 = c * u ; PE restream accumulates s
                p2 = qp.tile([128, 4096], BF16, tag="q")
                ccb = (
                    cc[:, :]
                    .rearrange("p (g j) -> p g j", g=8)
                    .unsqueeze(2)
                    .to_broadcast([128, 8, 16, 32])
                )
                nc.vector.tensor_mul(
                    p2[:, :].rearrange("p (g t j) -> p g t j", g=8, t=16), usl, ccb
                )
                for gl in range(8):
                    g = ch * 8 + gl
                    nc.tensor.matmul(
                        sps[:, :],
                        lhsT=ones[:, :],
                        rhs=p2[:, gl * 512 : (gl + 1) * 512],
                        start=(g == 0),
                        stop=(g == G - 1),
                    )

            for ch in range(9):
                if ch < 8:
                    front(ch)
                if ch >= 1:
                    back(ch - 1)

            v_r = _allreduce_squash(tc, dp, sp, sps, r=r, single=single)

        nc.sync.dma_start(outv, v_r[:, :])


def _allreduce_squash(tc, dp, sp, s_psum, r, single=False):
    """AllReduce s [32,512] across cores, then v = squash(s).

    s_psum is the PSUM accumulator.  Returns the v tile (bf16 for r<2,
    f32 for the final r=2).
    """
    nc = tc.nc
    spart = sp.tile([32, 512], F32, tag="s_part")
    nc.scalar.copy(spart[:, :], s_psum[:, :])
    ccin = dp.tile([32, 512], F32, tag=f"ccin{r}")
    ccout = dp.tile([32, 512], F32, tag=f"ccout{r}")
    nc.sync.dma_start(ccin[:, :], spart[:, :])
    if single:
        nc.sync.dma_start(ccout[:, :], ccin[:, :])
    else:
        nc.gpsimd.collective_compute(
            "AllReduce",
            mybir.AluOpType.add,
            replica_groups=[list(range(NCORES))],
            ins=[ccin[:, :].opt()],
            outs=[ccout[:, :].opt()],
        )
    s = sp.tile([32, 512], F32, tag="s_full")
    nc.sync.dma_start(s[:, :], ccout[:, :])

    # squash: v = s * (|s|^2/(1+|s|^2)/sqrt(|s|^2+eps)) per (b,j), |.| over t
    sq = sp.tile([32, 512], F32, tag="sq")
    nc.vector.tensor_mul(sq[:, :], s[:, :], s[:, :])
    ssq = sp.tile([32, 32], F32, tag="ssq")
    nc.vector.tensor_reduce(
        ssq[:, :],
        sq[:, :].rearrange("p (t j) -> p j t", t=16),
        mybir.AxisListType.X,
        mybir.AluOpType.add,
    )
    t1 = sp.tile([32, 32], F32, tag="t1")
    nc.vector.tensor_scalar_add(t1[:, :], ssq[:, :], 1.0)
    r1 = sp.tile([32, 32], F32, tag="r1")
    nc.vector.reciprocal(r1[:, :], t1[:, :])
    ssqe = sp.tile([32, 32], F32, tag="ssqe")
    nc.vector.tensor_scalar_add(ssqe[:, :], ssq[:, :], EPS)
    t2 = sp.tile([32, 32], F32, tag="t2")
    nc.scalar.activation(
        t2[:, :], ssqe[:, :], mybir.ActivationFunctionType.Sqrt, bias=0.0
    )
    r2 = sp.tile([32, 32], F32, tag="r2")
    nc.vector.reciprocal(r2[:, :], t2[:, :])
    sc = sp.tile([32, 32], F32, tag="sc")
    nc.vector.tensor_mul(sc[:, :], ssq[:, :], r1[:, :])
    nc.vector.tensor_mul(sc[:, :], sc[:, :], r2[:, :])
    v = sp.tile([32, 512], F32 if r == 2 else BF16, tag=f"v{r}")
    nc.vector.tensor_mul(
        v[:, :].rearrange("p (t j) -> p t j", t=16),
        s[:, :].rearrange("p (t j) -> p t j", t=16),
        sc[:, :].unsqueeze(1).to_broadcast([32, 16, 32]),
    )
    return v


@functools.lru_cache(maxsize=2)
def _get_nc(single=False):
    return _build_program(single=single)


def _prep_inputs(inputs, W):
    """Build per-core input maps (host-side layout only)."""
    inputs = np.asarray(inputs, dtype=np.float32)
    W = np.asarray(W, dtype=np.float32)
    W0 = W[0]  # [J, I, T, D]

    # delta_b ones [K=(i_sub 4, b 32), M=(b' 32)]
    ones = np.zeros((4, 32, 32), dtype=np.float32)
    for b in range(32):
        ones[:, b, b] = 1.0
    ones = ones.reshape(128, 32).astype(ml_dtypes.bfloat16)

    in_maps = []
    for c in range(NCORES):
        isl = slice(c * ILOC, (c + 1) * ILOC)
        ws = W0[:, isl]  # [J, 256, T, D]
        # wt[p, (gl, i_sub, d), (t, j)] ; i = 8p + 4gl + i_sub
        A = ws.transpose(1, 3, 2, 0)  # [i, d, t, j]
        A = A.reshape(NWT, 2, 4, D, T, J)  # p, gl, i_sub, d, t, j
        wtc = np.ascontiguousarray(A.reshape(NWT, 128, T * J)).astype(
            ml_dtypes.bfloat16
        )

        xs = inputs[:, isl]  # [b, 256, d]
        xt = xs.transpose(1, 2, 0)  # [i, d, b]
        xt5 = xt.reshape(NWT, 2, 4, D, B)  # p, gl, i_sub, d, b
        xdc = np.zeros((NWT, 2, 4, D, 4, B), dtype=np.float32)
        ar = np.arange(4)
        # advanced indexing: result axes [i_sub, p, gl, d, b]
        xdc[:, :, ar, :, ar, :] = xt5.transpose(2, 0, 1, 3, 4)
        xdc = np.ascontiguousarray(xdc.reshape(NWT, 128, 128)).astype(
            ml_dtypes.bfloat16
        )

        # s0 weights: x/J stacked over the full (gl, i_sub, d) partition
        xsc = np.ascontiguousarray((xt5 / float(J)).reshape(NWT, 128, B)).astype(
            ml_dtypes.bfloat16
        )

        in_maps.append({"wt": wtc, "xd": xdc, "xs0": xsc, "onesdb": ones})
    return in_maps


def kernel(inputs, W):
    import concourse.bass_utils as bass_utils

    nc = _get_nc()
    in_maps = _prep_inputs(inputs, W)
    res = bass_utils.run_bass_kernel_spmd(nc, in_maps, list(range(NCORES)))
    v = np.asarray(res.results[0]["outv"])  # [32, 512] = [b, (t, j)]
    return np.ascontiguousarray(
        v.reshape(B, T, J).transpose(0, 2, 1)
    ).astype(np.float32)
